# revision 59
# baseline (speedup 1.0000x reference)
"""Trainium2 Bass kernel for EASSA attention (8-core SPMD).

The reference module's state machine provably collapses: the create
score is `best - lam` with `lam = 1/max(budget, 1e-6) > 0`, so it can
never exceed `best` and a new state is created only when none exists
(t=0). A single state therefore accumulates the running mean of V, the
softmax over one valid state is exactly one-hot, and the attention
output is the cumulative mean of V. For the whole module:

    out[b, s, :] = (cumsum_s(x[b]) / (s+1)) @ (wv @ wo) + (bv @ wo + bo)

Q/K projections and the energy controller cannot affect the output.

Sharding: 8 lanes = (batch b in 0..3) x (sequence half h in 0..1),
uniform SPMD program. Cores owning a second half receive the first half
as input `xp` and fold its column-sum in as a scan prefix (first-half
cores receive zeros, keeping the program uniform).

Per-core pipeline (fp32, projections in float32r = 1 PE pass/row):
  Phase 1 (overlaps all DMA): per 128-token block, feature-major local
    cumsum C.T = matmul(lhsT=X_chunk, rhs=U_tri) on PE, rounded
    PSUM->SBUF, plus a prefix-independent local carry table (DVE).
  Prefix: xp streamed in 1 MiB quads, tree-reduced on DVE, column-sum
    via ones-matmuls.
  Phase 2: fold global prefix into each block (per-partition adds on
    DVE/ACT), project Y = sum_j C.T_j.T @ W_j (PE, fp32r, token-major
    PSUM out), optional rank-1 bias, scale by 1/(s+1) during the
    PSUM->SBUF copy, store in 2-block batches.
W = wv @ wo and c = bv @ wo + bo are built on-device (PE transposes of
wv blocks + fp32r matmuls) while phase 1 runs.

DMAs are batched (descriptor generation costs ~0.65us per issue on the
sequencers) and spread across the sync and gpsimd queues; weight/const
loads keep their descriptor stalls off the in-order ACT queue.
"""

from contextlib import ExitStack

import numpy as np

import concourse.bacc as bacc
import concourse.tile as tile
from concourse import mybir
from concourse.bass_utils import run_bass_kernel_spmd

F32 = mybir.dt.float32
P = 128          # partitions / tokens per block
D = 512          # model dim
HALF = 2048      # tokens per core
NBLK = HALF // P # 16
NCH = D // P     # 4 feature chunks
N_CORES = 8


def build_nc(with_bias=True, r_proj=True):
    """Build the (uniform SPMD) Bass program for one core.

    with_bias: emit the (bv @ wo + bo) bias path. Skipped when the host
    sees all-zero biases (then the path is numerically a no-op).
    r_proj: run the 512-wide projection matmuls in float32r (1 cy/row
    vs fp32's 4; ~1.5e-4 matmul rel err measured on HW).
    """
    nc = bacc.Bacc("TRN2", target_bir_lowering=False, debug=False)
    # fp32r operands must be written pre-rounded by a compute op (BIR
    # verifier checkMatmultFP32r), so projection-side tiles get F32R
    # dtype and their producers (DVE/ACT) round on write.
    F32R = mybir.dt.float32r if r_proj else F32

    xin = nc.dram_tensor("xin", [HALF, D], F32, kind="ExternalInput").ap()
    xp = nc.dram_tensor("xp", [HALF, D], F32, kind="ExternalInput").ap()
    wv = nc.dram_tensor("wv", [D, D], F32, kind="ExternalInput").ap()
    wo = nc.dram_tensor("wo", [D, D], F32, kind="ExternalInput").ap()
    bvc = nc.dram_tensor("bvc", [P, NCH], F32, kind="ExternalInput").ap()
    bo_row = nc.dram_tensor("bo_row", [1, D], F32, kind="ExternalInput").ap()
    u_tri = nc.dram_tensor("u_tri", [P, P], F32, kind="ExternalInput").ap()
    ident = nc.dram_tensor("ident", [P, P], F32, kind="ExternalInput").ap()
    ones_col = nc.dram_tensor("ones_col", [P, 1], F32, kind="ExternalInput").ap()
    counts_row = nc.dram_tensor("counts_row", [1, HALF], F32, kind="ExternalInput").ap()
    invs_cols = nc.dram_tensor("invs_cols", [P, NBLK], F32, kind="ExternalInput").ap()
    out = nc.dram_tensor("out", [HALF, D], F32, kind="ExternalOutput").ap()

    with tile.TileContext(nc) as tc, ExitStack() as ctx:
        consts = ctx.enter_context(tc.tile_pool(name="consts", bufs=1))

        # constants + weights on the GpSimd SWDGE queue: keeps their
        # descriptor-gen/ring stalls off the in-order ACT queue
        u_sb = consts.tile([P, P], F32, tag="u")
        nc.gpsimd.dma_start(u_sb[:], u_tri[:])
        invs_sb = consts.tile([P, NBLK], F32, tag="invs")
        nc.gpsimd.dma_start(invs_sb[:], invs_cols[:])

        wv_sb = consts.tile([P, NCH * D], F32, tag="wv")
        wo_sb = consts.tile([P, NCH * D], F32, tag="wo")
        nc.gpsimd.dma_start(
            wv_sb[:].rearrange("p (i m) -> p i m", m=D),
            wv.rearrange("(i p) m -> p i m", p=P),
        )
        nc.gpsimd.dma_start(
            wo_sb[:].rearrange("p (i m) -> p i m", m=D),
            wo.rearrange("(i p) m -> p i m", p=P),
        )
        id_sb = consts.tile([P, P], F32, tag="id")
        nc.gpsimd.dma_start(id_sb[:], ident[:])
        onec_sb = consts.tile([P, 1], F32, tag="onec")
        nc.gpsimd.dma_start(onec_sb[:], ones_col[:])
        bvc_sb = consts.tile([P, NCH], F32, tag="bvc")
        nc.gpsimd.dma_start(bvc_sb[:], bvc[:])
        bor_sb = consts.tile([1, D], F32, tag="bor")
        nc.gpsimd.dma_start(bor_sb[:], bo_row[:])
        counts_sb = consts.tile([1, HALF], F32, tag="counts")
        nc.gpsimd.dma_start(counts_sb[:], counts_row[:])

        wvt_sb = consts.tile([P, NCH * D], F32R, tag="wvt")
        w_sb = consts.tile([P, NCH * D], F32R, tag="w")
        wo_r = consts.tile([P, NCH * D], F32R, tag="wo_r")
        c_sb = consts.tile([1, D], F32, tag="c")
        p_sb = consts.tile([P, NCH], F32, tag="p")

        pool_pp = ctx.enter_context(tc.tile_pool(name="psum_pp", bufs=1, space="PSUM"))
        pool_pt = ctx.enter_context(tc.tile_pool(name="psum_pt", bufs=1, space="PSUM"))
        pool_pw = ctx.enter_context(tc.tile_pool(name="psum_pw", bufs=1, space="PSUM"))
        pool_pc = ctx.enter_context(tc.tile_pool(name="psum_pc", bufs=1, space="PSUM"))
        xin_pool = ctx.enter_context(tc.tile_pool(name="xin", bufs=1))
        cts_pool = ctx.enter_context(tc.tile_pool(name="cts", bufs=1))
        y_pool = ctx.enter_context(tc.tile_pool(name="y", bufs=1))
        psum_ct = ctx.enter_context(tc.tile_pool(name="psum_ct", bufs=2, space="PSUM"))
        psum_y = ctx.enter_context(tc.tile_pool(
            name="psum_y", bufs=2 if with_bias else 3, space="PSUM"))

        # Phase 1 (overlaps the xp prefix read): local cumsum of every
        # block, rounded to SBUF, plus an xp-independent local carry
        # table p_all[:, 4b+j] = sum_{m<b} colsum(block m, chunk j).
        p_all = consts.tile([P, NCH * NBLK], F32, tag="p_all")
        p_tot = consts.tile([P, NCH * NBLK], F32, tag="p_tot")
        nc.vector.memset(p_all[:, 0:NCH], 0.0)
        cts_tiles = []
        xinv = xin.rearrange("(n p) d -> p n d", p=P)
        xq_tiles = []
        for qi in range(8):
            xq = xin_pool.tile([P, 2 * D], F32, tag=f"xq{qi}", name=f"xq{qi}")
            nc.sync.dma_start(
                xq[:].rearrange("p (n d) -> p n d", d=D),
                xinv[:, 2 * qi:2 * (qi + 1), :],
            )
            xq_tiles.append(xq)
        for blk in range(NBLK):
            xt = xq_tiles[blk // 2]
            xoff = (blk % 2) * D

            # feature-major local cumsum: pct[:, j*128+s] = sum_{tau<=s} x[tau, j*128+p]
            pct = psum_ct.tile([P, D], F32, tag="pct")
            for j in range(NCH):
                nc.tensor.matmul(
                    pct[:, j * P:(j + 1) * P],
                    lhsT=xt[:, xoff + j * P:xoff + (j + 1) * P],
                    rhs=u_sb[:],
                    start=True,
                    stop=True,
                )
            # rounding copy PSUM -> SBUF (fp32r for the projection)
            cts = cts_pool.tile([P, D], F32R, tag=f"cts{blk}", name=f"cts{blk}")
            if blk % 2 == 0:
                nc.scalar.copy(cts[:], pct[:])
            else:
                nc.vector.tensor_copy(cts[:], pct[:])
            cts_tiles.append(cts)
            # local carry chain (exact fp32, from PSUM last-token cols)
            if blk < NBLK - 1:
                nc.vector.tensor_add(
                    p_all[:, (blk + 1) * NCH:(blk + 2) * NCH],
                    p_all[:, blk * NCH:(blk + 1) * NCH],
                    pct[:, P - 1::P],
                )

        # xp prefix bytes (share the wire with xin; only needed by
        # phase 2), pairwise tree-reduce on DVE.
        QW = 4 * D
        xpq = [
            consts.tile([P, QW], F32, tag=f"xpq{i}", name=f"xpq{i}")
            for i in range(4)
        ]
        xpv = xp.rearrange("(n p) d -> p n d", p=P)
        for i in range(4):
            nc.sync.dma_start(
                xpq[i][:].rearrange("p (n d) -> p n d", d=D),
                xpv[:, 4 * i:4 * (i + 1), :],
            )
        # per-quad fold to [128, 512] as each quad arrives, then a
        # short cross-quad chain (keeps the post-landing tail tiny)
        for i in range(4):
            nc.vector.tensor_add(
                xpq[i][:, 0:2 * D], xpq[i][:, 0:2 * D], xpq[i][:, 2 * D:4 * D]
            )
            nc.vector.tensor_add(
                xpq[i][:, 0:D], xpq[i][:, 0:D], xpq[i][:, D:2 * D]
            )
        nc.vector.tensor_add(xpq[0][:, 0:D], xpq[0][:, 0:D], xpq[1][:, 0:D])
        nc.vector.tensor_add(xpq[2][:, 0:D], xpq[2][:, 0:D], xpq[3][:, 0:D])
        xpacc = consts.tile([P, D], F32, tag="xpacc")
        nc.vector.tensor_add(xpacc[:], xpq[0][:, 0:D], xpq[2][:, 0:D])

        # prefix P[q, j] = sum_tau xp[tau, j*128+q]
        pp = pool_pp.tile([P, NCH], F32, tag="pp")
        for j in range(NCH):
            nc.tensor.matmul(
                pp[:, j:j + 1],
                lhsT=xpacc[:, j * P:(j + 1) * P],
                rhs=onec_sb[:],
                start=True,
                stop=True,
            )
        nc.vector.tensor_copy(p_sb[:], pp[:])

        # W-build after phase 1 (its results gate only phase 2, and its
        # ACT copies must not sit ahead of phase-1's cts copies in the
        # in-order ACT queue).
        nc.scalar.copy(wo_r[:], wo_sb[:])
            # transpose wv blocks: wvt_sb[:, j*D + i*128 + a] = wv[i*128+a, j*128+p]
            for j in range(NCH):
                pt = pool_pt.tile([P, D], F32, tag="pt")
                for i in range(NCH):
                    nc.tensor.transpose(
                        pt[:, i * P:(i + 1) * P],
                        wv_sb[:, i * D + j * P:i * D + (j + 1) * P],
                        id_sb[:],
                    )
                nc.scalar.copy(wvt_sb[:, j * D:(j + 1) * D], pt[:])

            # W rows chunk i: W[i*128+a, n] = sum_j wvT[j][.., a] @ wo[j][.., n]
            for i in range(NCH):
                pw = pool_pw.tile([P, D], F32, tag="pw")
                for j in range(NCH):
                    nc.tensor.matmul(
                        pw[:],
                        lhsT=wvt_sb[:, j * D + i * P:j * D + (i + 1) * P],
                        rhs=wo_r[:, j * D:(j + 1) * D],
                        start=(j == 0),
                        stop=(j == NCH - 1),
                    )
                nc.scalar.copy(w_sb[:, i * D:(i + 1) * D], pw[:])

            if with_bias:
                # c = bv @ wo + bo
                pc = pool_pc.tile([1, D], F32, tag="pc")
                for j in range(NCH):
                    nc.tensor.matmul(
                        pc[:],
                        lhsT=bvc_sb[:, j:j + 1],
                        rhs=wo_sb[:, j * D:(j + 1) * D],
                        start=(j == 0),
                        stop=(j == NCH - 1),
                    )
                nc.vector.tensor_add(c_sb[:], pc[:], bor_sb[:])

        outv = out.rearrange("(n p) d -> p n d", p=P)
        # Phase 2 (needs the xp prefix in p_sb): fold global prefix into
        # each block, project, scale, store.
        for blk in range(NBLK):
            cts = cts_tiles[blk]
            nc.vector.tensor_add(
                p_tot[:, blk * NCH:(blk + 1) * NCH],
                p_all[:, blk * NCH:(blk + 1) * NCH],
                p_sb[:],
            )
            for j in range(NCH):
                sl = cts[:, j * P:(j + 1) * P]
                sc = p_tot[:, blk * NCH + j:blk * NCH + j + 1]
                if j < 2:
                    nc.vector.tensor_scalar_add(sl, sl, sc)
                else:
                    nc.scalar.activation(
                        sl, sl, mybir.ActivationFunctionType.Identity, bias=sc
                    )

            # projection: Y[s, n] = sum_j cts_j[.., s].T @ W_j[.., n] + counts*c
            py = psum_y.tile([P, D], F32, tag="py")
            for j in range(NCH):
                nc.tensor.matmul(
                    py[:],
                    lhsT=cts[:, j * P:(j + 1) * P],
                    rhs=w_sb[:, j * D:(j + 1) * D],
                    start=(j == 0),
                    stop=(j == NCH - 1) and not with_bias,
                )
            if with_bias:
                nc.tensor.matmul(
                    py[:],
                    lhsT=counts_sb[:, blk * P:(blk + 1) * P],
                    rhs=c_sb[:],
                    start=False,
                    stop=True,
                )

            # scale by 1/(s+1) during PSUM->SBUF copy into a quad
            # staging tile; one store DMA per 4 blocks
            if blk % 2 == 0:
                yq = y_pool.tile([P, 2 * D], F32, tag=f"yq{blk // 2}",
                                 name=f"yq{blk // 2}")
            ysl = yq[:, (blk % 2) * D:(blk % 2 + 1) * D]
            if blk % 2 == 0:
                nc.scalar.mul(ysl, py[:], invs_sb[:, blk:blk + 1])
            else:
                nc.vector.tensor_scalar_mul(ysl, py[:], invs_sb[:, blk:blk + 1])
            if blk % 2 == 1:
                qi = blk // 2
                nc.sync.dma_start(
                    outv[:, 2 * qi:2 * (qi + 1), :],
                    yq[:].rearrange("p (n d) -> p n d", d=D),
                )

    nc.compile()
    return nc


def make_in_maps(x, wv, bv, wo, bo):
    B, S, Dm = x.shape
    assert (B, S, Dm) == (4, 4096, 512)
    x = np.ascontiguousarray(np.asarray(x, dtype=np.float32))
    wv = np.ascontiguousarray(np.asarray(wv, dtype=np.float32))
    wo = np.ascontiguousarray(np.asarray(wo, dtype=np.float32))
    bv = np.asarray(bv, dtype=np.float32)
    bo = np.asarray(bo, dtype=np.float32)

    u_tri = np.triu(np.ones((P, P), dtype=np.float32))
    ident = np.eye(P, dtype=np.float32)
    ones_col = np.ones((P, 1), dtype=np.float32)
    bvc = np.ascontiguousarray(bv.reshape(NCH, P).T)
    bo_row = np.ascontiguousarray(bo.reshape(1, D))
    zeros_half = np.zeros((HALF, D), dtype=np.float32)

    in_maps = []
    for c in range(N_CORES):
        b, h = c // 2, c % 2
        off = h * HALF
        counts = np.arange(off + 1, off + HALF + 1, dtype=np.float32)
        in_maps.append({
            "xin": np.ascontiguousarray(x[b, off:off + HALF, :]),
            "xp": np.ascontiguousarray(x[b, 0:HALF, :]) if h == 1 else zeros_half,
            "wv": wv,
            "wo": wo,
            "bvc": bvc,
            "bo_row": bo_row,
            "u_tri": u_tri,
            "ident": ident,
            "ones_col": ones_col,
            "counts_row": np.ascontiguousarray(counts.reshape(1, HALF)),
            "invs_cols": np.ascontiguousarray((1.0 / counts).reshape(NBLK, P).T),
        })
    return in_maps


_NC_CACHE = {}


def run(inputs, trace=False, trace_cores=None):
    """Shard, run on 8 cores, gather. Returns (out, BassKernelResults)."""
    with_bias = bool(
        np.any(np.asarray(inputs["bv"])) or np.any(np.asarray(inputs["bo"]))
    )
    key = ("nc", with_bias)
    if key not in _NC_CACHE:
        _NC_CACHE[key] = build_nc(with_bias=with_bias)
    nc = _NC_CACHE[key]
    in_maps = make_in_maps(
        inputs["x"], inputs["wv"], inputs["bv"], inputs["wo"], inputs["bo"]
    )
    res = run_bass_kernel_spmd(
        nc, in_maps, list(range(N_CORES)), trace=trace, trace_cores=trace_cores
    )
    x = np.asarray(inputs["x"])
    out = np.empty((4, 4096, 512), dtype=np.float32)
    for c in range(N_CORES):
        b, h = c // 2, c % 2
        out[b, h * HALF:(h + 1) * HALF, :] = res.results[c]["out"]
    return out, res


def kernel(**inputs):
    out, _ = run(inputs, trace=False)
    return out
